# revision 2
# baseline (speedup 1.0000x reference)
"""B2Bsqrt-TANDEM LSTM kernel for Trainium2 (8 NeuronCores, data-parallel).

v2 rework of the baseline: same dual-streamed bf16 pair matmuls (full
128-col PE width via tile_position column groups), but restructured so
the PE never idles (the baseline lost ~0.8ms/run to a 3us PE gap per
step plus the HAM re-throttle it caused):

- gate order [o, i, f, ct]: sigmoids retire early, the c~ gate lands
  last so its sqrt chain overlaps the xz prefill of step t+1
- ALL FOUR gates of step t+1 are prefilled (xz part) right after the
  hU accumulation of step t, so the PE has ~6.8us of work covering the
  ~4us ACT/DVE tail of the recurrence
- scalar-engine diet: abs via DVE abs_max, sign(c) applied via gpsimd
  bitwise AND/XOR on bf16 bits, LN stats via one DVE bn_stats, and the
  whole LayerNorm/FC finalize moved after the loop (ACT does only
  3 sigmoids + 2 sqrts + 1 dummy per step; the dummy sqrt forces the
  act-table load into the idle window before z_ct is ready)
- h transposed with 4 full 128x128 PE transposes (fold pairs) into one
  PSUM tile + a single DVE copy
"""

import sys

sys.path.insert(0, "/opt/trn_rl_repo")

import numpy as np
import ml_dtypes

import concourse.bass as bass
import concourse.mybir as mybir
import concourse.tile as tile
from concourse import bacc
from concourse.bass_utils import run_bass_kernel_spmd
from concourse.masks import make_identity
from concourse.tile_rust import add_dep_helper

AF = mybir.ActivationFunctionType
OP = mybir.AluOpType
BF16 = mybir.dt.bfloat16
F32 = mybir.dt.float32
U16 = mybir.dt.uint16

N_CORES = 8
B_FULL = 512
BL = B_FULL // N_CORES  # 64 batch rows per core
T_FULL = 100
H = 1024
G4 = 4 * H  # 4096
C = 10
KC = H // 128  # 8 contraction chunks
LN_EPS = 1e-5

GATE_ORDER = [2, 0, 1, 3]  # o, i, f, ct


def emit(ctx, nc, tc, T, with_bias, u_vec, vb_vec):
    sing = ctx.enter_context(tc.tile_pool(name="sing", bufs=1))
    xt_pool = ctx.enter_context(tc.tile_pool(name="xt", bufs=3))
    ht_pool = ctx.enter_context(tc.tile_pool(name="ht", bufs=2))
    gp = ctx.enter_context(tc.tile_pool(name="gp", bufs=1))
    sp = ctx.enter_context(tc.tile_pool(name="sp", bufs=2))
    zp = ctx.enter_context(tc.tile_pool(name="zp", bufs=4, space="PSUM"))
    tp = ctx.enter_context(tc.tile_pool(name="tp", bufs=2, space="PSUM"))
    fp = ctx.enter_context(tc.tile_pool(name="fp", bufs=1, space="PSUM"))
    lp = ctx.enter_context(tc.tile_pool(name="lp", bufs=1, space="PSUM"))

    dW = nc.dram_tensor("Wn", [KC, 128, G4], BF16, kind="ExternalInput")
    dU = nc.dram_tensor("Un", [KC, 128, G4], BF16, kind="ExternalInput")
    dX = nc.dram_tensor("xT", [T, KC, 128, BL], BF16, kind="ExternalInput")
    dGW = nc.dram_tensor("gw", [KC, 128, C], BF16, kind="ExternalInput")
    if with_bias:
        dBB = nc.dram_tensor("bb", [128, G4], BF16, kind="ExternalInput")
    dOUT = nc.dram_tensor("out", [BL, T * C], F32, kind="ExternalOutput")

    # --- resident weights / constants ---
    W_sb = sing.tile([128, KC, G4], BF16)
    nc.sync.dma_start(W_sb[:], dW.rearrange("k p n -> p k n"))
    U_sb = sing.tile([128, KC, G4], BF16)
    nc.sync.dma_start(U_sb[:], dU.rearrange("k p n -> p k n"))
    gw_sb = sing.tile([128, KC, C], BF16)
    nc.sync.dma_start(gw_sb[:], dGW.rearrange("k p c -> p k c"))
    if with_bias:
        bb_sb = sing.tile([128, G4], BF16)
        nc.sync.dma_start(bb_sb[:], dBB[:])
        ones_col = sing.tile([128, BL], BF16)
        nc.vector.memset(ones_col[:], 0.0)
        nc.vector.memset(ones_col[0:1, :], 1.0)

    id128 = sing.tile([128, 128], BF16)
    make_identity(nc, id128[:])
    id10 = sing.tile([C, C], F32)
    make_identity(nc, id10[:])

    eps_sb = sing.tile([BL, 1], F32)
    nc.vector.memset(eps_sb[:], LN_EPS)
    dum_in = sing.tile([BL, 1], F32)
    nc.vector.memset(dum_in[:], 1.0)

    msk = sing.tile([128, 256], U32)
    nc.vector.memset(msk[:], 0x80008000)  # bf16 sign-bit mask, packed pairs
    mskA = sing.tile([128, 512], U32)
    nc.vector.memset(mskA[:], 0x7FFFFFFF)  # f32 abs mask
    mskB = sing.tile([128, 256], U32)
    nc.vector.memset(mskB[:], 0x7FFF7FFF)  # bf16 abs mask, packed pairs

    c_st = sing.tile([128, 512], BF16)
    nc.vector.memset(c_st[:], 0.0)

    logitsT = sing.tile([C, T, BL], F32)
    stats = sing.tile([128, T, 6], F32)

    act_chain = [None]

    def act(*args, **kwargs):
        inst = nc.scalar.activation(*args, **kwargs)
        if act_chain[0] is not None:
            add_dep_helper(inst.ins, act_chain[0].ins, False, "act order")
        act_chain[0] = inst
        return inst

    xts = {}

    def get_xt(t):
        if t not in xts:
            xt = xt_pool.tile([128, KC, BL], BF16, tag="xt")
            nc.sync.dma_start(xt[:], dX[t].rearrange("k p b -> p k b"))
            xts[t] = xt
        return xts[t]

    def mm_pair(z_ps, lhsT_k, rhs, g, k, start, stop=False):
        """Col-tiled pair: half A -> psum partitions 0:64, half B -> 64:128."""
        nsA = slice(g * 1024, g * 1024 + 512)
        nsB = slice(g * 1024 + 512, (g + 1) * 1024)
        nc.tensor.matmul(z_ps[0:BL, :], lhsT_k, rhs[:, k, nsA],
                         start=start, stop=stop, tile_position=(0, 0))
        nc.tensor.matmul(z_ps[BL:128, :], lhsT_k, rhs[:, k, nsB],
                         start=start, stop=stop, tile_position=(0, 64),
                         skip_group_check=True)

    def emit_xz(t, g, with_stop):
        z_ps = zp.tile([128, 512], F32, tag="z")
        xt = get_xt(t)
        last = KC - 1
        for k in range(KC):
            st = with_stop and (k == last) and not with_bias
            mm_pair(z_ps, xt[:, k, :], W_sb, g, k, start=(k == 0), stop=st)
        if with_bias:
            nsA = slice(g * 1024, g * 1024 + 512)
            nsB = slice(g * 1024 + 512, (g + 1) * 1024)
            nc.tensor.matmul(z_ps[0:BL, :], ones_col[:], bb_sb[:, nsA],
                             start=False, stop=with_stop, tile_position=(0, 0))
            nc.tensor.matmul(z_ps[BL:128, :], ones_col[:], bb_sb[:, nsB],
                             start=False, stop=with_stop, tile_position=(0, 64),
                             skip_group_check=True)
        return z_ps

    def hT_slice(hT, k):
        return hT[:, k, 0:BL] if k < 4 else hT[:, k - 4, BL:128]

    pending = {}
    hT_prev = None

    for t in range(T):
        sig_o = gp.tile([128, 512], BF16, tag="sig_o")
        sig_i = gp.tile([128, 512], BF16, tag="sig_i")
        sig_f = gp.tile([128, 512], BF16, tag="sig_f")
        sg3 = gp.tile([128, 512], BF16, tag="sg3")
        a3u = gp.tile([128, 512], U32, tag="a3u")
        tmp1 = gp.tile([128, 512], BF16, tag="tmp1")

        z3_ps = None
        # --- PE: recurrent accumulation (h @ U) on prefilled xz ---
        for g in GATE_ORDER:
            z_ps = pending.pop(g, None)
            if z_ps is None:
                z_ps = emit_xz(t, g, with_stop=True)  # t == 0 only
            if hT_prev is not None:
                for k in range(KC):
                    mm_pair(z_ps, hT_slice(hT_prev, k), U_sb, g, k,
                            start=False, stop=(k == KC - 1))
            if g == 2:
                act(sig_o[:], z_ps[:], AF.Sigmoid)
            elif g == 0:
                act(sig_i[:], z_ps[:], AF.Sigmoid)
            elif g == 1:
                act(sig_f[:], z_ps[:], AF.Sigmoid)
                # f * c_prev as soon as f is out
                nc.vector.tensor_tensor(tmp1[:], sig_f[:], c_st[:], OP.mult)
                # dummy sqrt-set op: forces the act-table switch to happen
                # now, in the ACT idle window, not on the z_ct critical path
                dum = sp.tile([BL, 1], F32, tag="dum")
                act(dum[:], dum_in[:], AF.Sqrt)
            else:
                z3_ps = z_ps
                # |z3| on DVE (bitwise clear of f32 sign), sign(z3) on ACT
                nc.vector.tensor_tensor(a3u[:], z_ps[:].bitcast(U32),
                                        mskA[:], OP.bitwise_and)
                act(sg3[:], z_ps[:], AF.Sign)

        # --- PE: prefill xz of ALL gates of t+1 (fills the ACT/DVE tail) ---
        if t + 1 < T:
            for g in GATE_ORDER:
                pending[g] = emit_xz(t + 1, g, with_stop=False)

        # --- c~ = sign(z3)*(sqrt(1+|z3|)-1); c = f*c + i*c~ ---
        s3 = gp.tile([128, 512], F32, tag="s3")
        act(s3[:], a3u[:].bitcast(F32), AF.Sqrt, bias=1.0)
        i_sg = gp.tile([128, 512], BF16, tag="i_sg")
        nc.gpsimd.tensor_tensor(i_sg[:], sig_i[:], sg3[:], OP.mult)
        s3m1 = gp.tile([128, 512], BF16, tag="s3m1")
        nc.vector.tensor_scalar(s3m1[:], s3[:], 1.0, None, OP.subtract)
        tmp2 = gp.tile([128, 512], BF16, tag="tmp2")
        nc.vector.tensor_tensor(tmp2[:], i_sg[:], s3m1[:], OP.mult)
        nc.vector.tensor_tensor(c_st[:], tmp1[:], tmp2[:], OP.add)

        # --- h = sig_o * sign(c) * (sqrt(1+|c|)-1) ---
        ac = gp.tile([128, 256], U32, tag="ac")
        nc.vector.tensor_tensor(ac[:], c_st[:].bitcast(U32), mskB[:],
                                OP.bitwise_and)
        sc = gp.tile([128, 512], F32, tag="sc")
        act(sc[:], ac[:].bitcast(BF16), AF.Sqrt, bias=1.0)
        cand = gp.tile([128, 256], U32, tag="cand")
        nc.vector.tensor_tensor(cand[:], c_st[:].bitcast(U32), msk[:],
                                OP.bitwise_and)
        osg = gp.tile([128, 256], U32, tag="osg")
        nc.vector.tensor_tensor(osg[:], cand[:], sig_o[:].bitcast(U32),
                                OP.bitwise_xor)
        scm1 = gp.tile([128, 512], BF16, tag="scm1")
        nc.vector.tensor_scalar(scm1[:], sc[:], 1.0, None, OP.subtract)
        h_bf = gp.tile([128, 512], BF16, tag="h_bf")
        nc.vector.tensor_tensor(h_bf[:], scm1[:], osg[:].bitcast(BF16),
                                OP.mult)

        # --- PE: transpose h (4 full 128x128 fold-pair transposes) ---
        t_all = tp.tile([128, 4, 128], BF16, tag="tall")
        for j in range(4):
            nc.tensor.transpose(t_all[:, j, :],
                                h_bf[:, j * 128:(j + 1) * 128], id128[:])
        hT = ht_pool.tile([128, 4, 128], BF16, tag="hT")
        nc.vector.tensor_copy(hT[:], t_all[:])

        # --- PE: FC raw logits (transposed): f_ps = gw.T @ hT ---
        f_ps = fp.tile([C, BL], F32, tag="fps")
        for k in range(KC):
            nc.tensor.matmul(f_ps[:], gw_sb[:, k, :], hT_slice(hT, k),
                             start=(k == 0), stop=(k == KC - 1))
        nc.vector.tensor_copy(logitsT[:, t, :], f_ps[:])

        # --- LN stats (mean/M2 per partition-half; merged post-loop) ---
        nc.vector.bn_stats(stats[:, t, :], h_bf[:])

        hT_prev = hT
        xts.pop(t, None)

    # ================= post-loop finalize =================
    # fold the upper partition half of the stats next to the lower half
    st_lo = sing.tile([BL, T, 6], F32)
    nc.sync.dma_start(st_lo[:], stats[BL:128, :, :])
    up = stats

    def f_tile(tag):
        return sp.tile([BL, T], F32, tag=tag, name=tag)

    # sum of the 4 group means and of the 4 squared means, sum of M2s
    msum = f_tile("msum")
    nc.vector.tensor_tensor(msum[:], up[0:BL, :, 1], up[0:BL, :, 4], OP.add)
    t_a = f_tile("t_a")
    nc.vector.tensor_tensor(t_a[:], st_lo[:, :, 1], st_lo[:, :, 4], OP.add)
    nc.vector.tensor_tensor(msum[:], msum[:], t_a[:], OP.add)

    def sq_add(acc, src, first):
        q = f_tile("q")
        nc.vector.tensor_tensor(q[:], src, src, OP.mult)
        if first:
            nc.vector.tensor_copy(acc[:], q[:])
        else:
            nc.vector.tensor_tensor(acc[:], acc[:], q[:], OP.add)

    msq = f_tile("msq")
    sq_add(msq, up[0:BL, :, 1], True)
    sq_add(msq, up[0:BL, :, 4], False)
    sq_add(msq, st_lo[:, :, 1], False)
    sq_add(msq, st_lo[:, :, 4], False)

    m2s = f_tile("m2s")
    nc.vector.tensor_tensor(m2s[:], up[0:BL, :, 2], up[0:BL, :, 5], OP.add)
    nc.vector.tensor_tensor(t_a[:], st_lo[:, :, 2], st_lo[:, :, 5], OP.add)
    nc.vector.tensor_tensor(m2s[:], m2s[:], t_a[:], OP.add)

    mu = f_tile("mu")
    nc.vector.tensor_scalar(mu[:], msum[:], 0.25, None, OP.mult)
    e2 = f_tile("e2")
    nc.vector.tensor_scalar(e2[:], m2s[:], 1.0 / H, None, OP.mult)
    nc.vector.tensor_scalar(t_a[:], msq[:], 0.25, None, OP.mult)
    nc.vector.tensor_tensor(e2[:], e2[:], t_a[:], OP.add)
    var = f_tile("var")
    nc.vector.tensor_tensor(t_a[:], mu[:], mu[:], OP.mult)
    nc.vector.tensor_tensor(var[:], e2[:], t_a[:], OP.subtract)
    sd = f_tile("sd")
    act(sd[:], var[:], AF.Sqrt, bias=eps_sb[:])
    rsig = f_tile("rsig")
    nc.vector.reciprocal(rsig[:], sd[:])
    murs = f_tile("murs")
    nc.vector.tensor_tensor(murs[:], mu[:], rsig[:], OP.mult)

    # raw logits back to batch-major (batches of 8 steps per PSUM bank)
    raw_bm = sing.tile([BL, T, C], F32)
    for t0 in range(0, T, 8):
        nb = min(8, T - t0)
        lt_ps = lp.tile([BL, 8, C], F32, tag="lt")
        for jj in range(nb):
            nc.tensor.transpose(lt_ps[:, jj, :], logitsT[:, t0 + jj, :],
                                id10[:])
        nc.vector.tensor_copy(raw_bm[:, t0:t0 + nb, :], lt_ps[:, 0:nb, :])

    out_bm = sing.tile([BL, T, C], F32)
    tb = f_tile("tb")
    for c in range(C):
        # out_c = raw_c * rsig - (u_c * mu*rsig - vb_c)
        nc.vector.tensor_scalar(tb[:], murs[:], float(u_vec[c]),
                                float(vb_vec[c]), OP.mult, OP.subtract)
        ta2 = f_tile("ta2")
        nc.vector.tensor_tensor(ta2[:], raw_bm[:, :, c], rsig[:], OP.mult)
        nc.vector.tensor_tensor(out_bm[:, :, c], ta2[:], tb[:], OP.subtract)

    nc.sync.dma_start(dOUT[:], out_bm[:].rearrange("b t c -> b (t c)"))


def build(T=T_FULL, with_bias=False, u_vec=None, vb_vec=None):
    from contextlib import ExitStack

    nc = bacc.Bacc("TRN2", target_bir_lowering=False)
    with tile.TileContext(nc) as tc:
        with ExitStack() as ctx:
            emit(ctx, nc, tc, T, with_bias, u_vec, vb_vec)
    nc.compile()
    return nc


def kernel(x, W, U, b, ln_g, ln_b, fc_w, fc_b, _T=None, _trace=False):
    x = np.asarray(x, dtype=np.float32)
    W = np.asarray(W, dtype=np.float32)
    U = np.asarray(U, dtype=np.float32)
    b = np.asarray(b, dtype=np.float32)
    ln_g = np.asarray(ln_g, dtype=np.float32)
    ln_b = np.asarray(ln_b, dtype=np.float32)
    fc_w = np.asarray(fc_w, dtype=np.float32)
    fc_b = np.asarray(fc_b, dtype=np.float32)

    T = _T or x.shape[1]
    with_bias = bool(np.any(b))

    W_all = np.concatenate([W[g] for g in range(4)], axis=1)  # (H, 4H)
    U_all = np.concatenate([U[g] for g in range(4)], axis=1)
    Wn = np.ascontiguousarray(
        W_all.reshape(KC, 128, G4)).astype(ml_dtypes.bfloat16)
    Un = np.ascontiguousarray(
        U_all.reshape(KC, 128, G4)).astype(ml_dtypes.bfloat16)
    gw = (ln_g[:, None] * fc_w).reshape(KC, 128, C).astype(ml_dtypes.bfloat16)
    u_vec = (ln_g @ fc_w).astype(np.float32)  # (C,)
    vb = (ln_b @ fc_w + fc_b).astype(np.float32)

    common = {"Wn": Wn, "Un": Un, "gw": gw}
    if with_bias:
        b_all = np.concatenate([b[g] for g in range(4)])  # (4H,)
        bb = np.zeros((128, G4), dtype=ml_dtypes.bfloat16)
        bb[0, :] = b_all.astype(ml_dtypes.bfloat16)
        common["bb"] = bb

    in_maps = []
    for ci in range(N_CORES):
        xc = x[ci * BL:(ci + 1) * BL, :T]           # (BL, T, H)
        xT = xc.transpose(1, 2, 0)                   # (T, H, BL)
        xT = np.ascontiguousarray(xT).reshape(T, KC, 128, BL)
        in_maps.append({"xT": xT.astype(ml_dtypes.bfloat16), **common})

    nc = build(T, with_bias, u_vec, vb)
    res = run_bass_kernel_spmd(nc, in_maps, core_ids=list(range(N_CORES)),
                               trace=_trace)
    out = np.concatenate(
        [r["out"].reshape(BL, T, C) for r in res.results], axis=0)
    if _trace:
        kernel.last_exec_time_ns = res.exec_time_ns
    return out


# revision 3
# speedup vs baseline: 1.1965x; 1.1965x over previous
"""B2Bsqrt-TANDEM LSTM kernel for Trainium2 (8 NeuronCores, data-parallel).

v2 rework of the baseline: same dual-streamed bf16 pair matmuls (full
128-col PE width via tile_position column groups), but restructured so
the PE never idles (the baseline lost ~0.8ms/run to a 3us PE gap per
step plus the HAM re-throttle it caused):

- gate order [o, i, f, ct]: sigmoids retire early, the c~ gate lands
  last so its sqrt chain overlaps the xz prefill of step t+1
- ALL FOUR gates of step t+1 are prefilled (xz part) right after the
  hU accumulation of step t, so the PE has ~6.8us of work covering the
  ~4us ACT/DVE tail of the recurrence
- scalar-engine diet: abs via DVE abs_max, sign(c) applied via gpsimd
  bitwise AND/XOR on bf16 bits, LN stats via one DVE bn_stats, and the
  whole LayerNorm/FC finalize moved after the loop (ACT does only
  3 sigmoids + 2 sqrts + 1 dummy per step; the dummy sqrt forces the
  act-table load into the idle window before z_ct is ready)
- h transposed with 4 full 128x128 PE transposes (fold pairs) into one
  PSUM tile + a single DVE copy
"""

import sys

sys.path.insert(0, "/opt/trn_rl_repo")

import numpy as np
import ml_dtypes

import concourse.bass as bass
import concourse.mybir as mybir
import concourse.tile as tile
from concourse import bacc
from concourse.bass_utils import run_bass_kernel_spmd
from concourse.masks import make_identity
from concourse.tile_rust import add_dep_helper

AF = mybir.ActivationFunctionType
OP = mybir.AluOpType
BF16 = mybir.dt.bfloat16
F32 = mybir.dt.float32
U16 = mybir.dt.uint16

N_CORES = 8
B_FULL = 512
BL = B_FULL // N_CORES  # 64 batch rows per core
T_FULL = 100
H = 1024
G4 = 4 * H  # 4096
C = 10
KC = H // 128  # 8 contraction chunks
LN_EPS = 1e-5

GATE_ORDER = [2, 0, 1, 3]  # o, i, f, ct


def emit(ctx, nc, tc, T, with_bias, u_vec, vb_vec):
    sing = ctx.enter_context(tc.tile_pool(name="sing", bufs=1))
    xt_pool = ctx.enter_context(tc.tile_pool(name="xt", bufs=3))
    ht_pool = ctx.enter_context(tc.tile_pool(name="ht", bufs=2))
    gp = ctx.enter_context(tc.tile_pool(name="gp", bufs=1))
    sp = ctx.enter_context(tc.tile_pool(name="sp", bufs=2))
    zp = ctx.enter_context(tc.tile_pool(name="zp", bufs=4, space="PSUM"))
    tp = ctx.enter_context(tc.tile_pool(name="tp", bufs=2, space="PSUM"))
    fp = ctx.enter_context(tc.tile_pool(name="fp", bufs=1, space="PSUM"))
    lp = ctx.enter_context(tc.tile_pool(name="lp", bufs=1, space="PSUM"))

    dW = nc.dram_tensor("Wn", [KC, 128, G4], BF16, kind="ExternalInput")
    dU = nc.dram_tensor("Un", [KC, 128, G4], BF16, kind="ExternalInput")
    dX = nc.dram_tensor("xT", [T, KC, 128, BL], BF16, kind="ExternalInput")
    dGW = nc.dram_tensor("gw", [KC, 128, C], BF16, kind="ExternalInput")
    if with_bias:
        dBB = nc.dram_tensor("bb", [128, G4], BF16, kind="ExternalInput")
    dOUT = nc.dram_tensor("out", [BL, T * C], F32, kind="ExternalOutput")

    # --- resident weights / constants (chunked so step 0 starts early) ---
    W_sb = sing.tile([128, KC, G4], BF16)
    for k in range(KC):
        nc.sync.dma_start(W_sb[:, k, :], dW[k])
    gw_sb = sing.tile([128, KC, C], BF16)
    nc.sync.dma_start(gw_sb[:], dGW.rearrange("k p c -> p k c"))
    U_sb = sing.tile([128, KC, G4], BF16)
    if with_bias:
        bb_sb = sing.tile([128, G4], BF16)
        nc.sync.dma_start(bb_sb[:], dBB[:])
        ones_col = sing.tile([128, BL], BF16)
        nc.vector.memset(ones_col[:], 0.0)
        nc.vector.memset(ones_col[0:1, :], 1.0)

    id128 = sing.tile([128, 128], BF16)
    make_identity(nc, id128[:])
    id10 = sing.tile([C, C], F32)
    make_identity(nc, id10[:])

    eps_sb = sing.tile([BL, 1], F32)
    nc.vector.memset(eps_sb[:], LN_EPS)
    dum_in = sing.tile([BL, 1], F32)
    nc.vector.memset(dum_in[:], 1.0)

    msk = sing.tile([128, 256], U32)
    nc.vector.memset(msk[:], 0x80008000)  # bf16 sign-bit mask, packed pairs
    mskA = sing.tile([128, 512], U32)
    nc.vector.memset(mskA[:], 0x7FFFFFFF)  # f32 abs mask
    mskB = sing.tile([128, 256], U32)
    nc.vector.memset(mskB[:], 0x7FFF7FFF)  # bf16 abs mask, packed pairs

    c_st = sing.tile([128, 512], BF16)
    nc.vector.memset(c_st[:], 0.0)

    logitsT = sing.tile([C, T, BL], F32)
    stats = sing.tile([128, T, 6], F32)

    act_chain = [None]

    def act(*args, **kwargs):
        inst = nc.scalar.activation(*args, **kwargs)
        if act_chain[0] is not None:
            add_dep_helper(inst.ins, act_chain[0].ins, False, "act order")
        act_chain[0] = inst
        return inst

    xts = {}

    def get_xt(t):
        if t not in xts:
            xt = xt_pool.tile([128, KC, BL], BF16, tag="xt")
            nc.sync.dma_start(xt[:], dX[t].rearrange("k p b -> p k b"))
            xts[t] = xt
        return xts[t]

    # x for the first two steps before the big U transfer hits the queue
    get_xt(0)
    get_xt(1)
    for k in range(KC):
        nc.sync.dma_start(U_sb[:, k, :], dU[k])

    def mm_pair(z_ps, lhsT_k, rhs, g, k, start, stop=False):
        """Col-tiled pair: half A -> psum partitions 0:64, half B -> 64:128."""
        nsA = slice(g * 1024, g * 1024 + 512)
        nsB = slice(g * 1024 + 512, (g + 1) * 1024)
        nc.tensor.matmul(z_ps[0:BL, :], lhsT_k, rhs[:, k, nsA],
                         start=start, stop=stop, tile_position=(0, 0))
        nc.tensor.matmul(z_ps[BL:128, :], lhsT_k, rhs[:, k, nsB],
                         start=start, stop=stop, tile_position=(0, 64),
                         skip_group_check=True)

    def emit_xz(t, g, with_stop):
        z_ps = zp.tile([128, 512], F32, tag="z")
        xt = get_xt(t)
        last = KC - 1
        for k in range(KC):
            st = with_stop and (k == last) and not with_bias
            mm_pair(z_ps, xt[:, k, :], W_sb, g, k, start=(k == 0), stop=st)
        if with_bias:
            nsA = slice(g * 1024, g * 1024 + 512)
            nsB = slice(g * 1024 + 512, (g + 1) * 1024)
            nc.tensor.matmul(z_ps[0:BL, :], ones_col[:], bb_sb[:, nsA],
                             start=False, stop=with_stop, tile_position=(0, 0))
            nc.tensor.matmul(z_ps[BL:128, :], ones_col[:], bb_sb[:, nsB],
                             start=False, stop=with_stop, tile_position=(0, 64),
                             skip_group_check=True)
        return z_ps

    def hT_slice(hT, k):
        return hT[:, k, 0:BL] if k < 4 else hT[:, k - 4, BL:128]

    # step 0 xz emitted k-outer across gates so compute starts after the
    # first W chunk + x0 land, overlapping the rest of the weight DMA
    pending = {}
    xt0 = get_xt(0)
    for g in GATE_ORDER:
        pending[g] = zp.tile([128, 512], F32, tag="z", name=f"z0g{g}")
    for k in range(KC):
        for g in GATE_ORDER:
            st = (k == KC - 1) and not with_bias
            mm_pair(pending[g], xt0[:, k, :], W_sb, g, k,
                    start=(k == 0), stop=st)
    if with_bias:
        for g in GATE_ORDER:
            nsA = slice(g * 1024, g * 1024 + 512)
            nsB = slice(g * 1024 + 512, (g + 1) * 1024)
            nc.tensor.matmul(pending[g][0:BL, :], ones_col[:], bb_sb[:, nsA],
                             start=False, stop=True, tile_position=(0, 0))
            nc.tensor.matmul(pending[g][BL:128, :], ones_col[:],
                             bb_sb[:, nsB], start=False, stop=True,
                             tile_position=(0, 64), skip_group_check=True)

    K_ORDER = [0, 4, 1, 5, 2, 6, 3, 7]  # each hT copy chunk unblocks two k
    hT_prev = None

    for t in range(T):
        sig_o = gp.tile([128, 512], BF16, tag="sig_o")
        sig_i = gp.tile([128, 512], BF16, tag="sig_i")
        sig_f = gp.tile([128, 512], BF16, tag="sig_f")
        sg3 = gp.tile([128, 512], BF16, tag="sg3")
        a3u = gp.tile([128, 512], U32, tag="a3u")
        tmp1 = gp.tile([128, 512], BF16, tag="tmp1")

        z3_ps = None
        # --- PE: recurrent accumulation (h @ U) on prefilled xz ---
        for g in GATE_ORDER:
            z_ps = pending.pop(g, None)
            if z_ps is None:
                z_ps = emit_xz(t, g, with_stop=True)  # t == 0 only
            if hT_prev is not None:
                for k in range(KC):
                    mm_pair(z_ps, hT_slice(hT_prev, k), U_sb, g, k,
                            start=False, stop=(k == KC - 1))
            if g == 2:
                act(sig_o[:], z_ps[:], AF.Sigmoid)
            elif g == 0:
                act(sig_i[:], z_ps[:], AF.Sigmoid)
            elif g == 1:
                act(sig_f[:], z_ps[:], AF.Sigmoid)
                # f * c_prev as soon as f is out
                nc.vector.tensor_tensor(tmp1[:], sig_f[:], c_st[:], OP.mult)
                # dummy sqrt-set op: forces the act-table switch to happen
                # now, in the ACT idle window, not on the z_ct critical path
                dum = sp.tile([BL, 1], F32, tag="dum")
                act(dum[:], dum_in[:], AF.Sqrt)
            else:
                z3_ps = z_ps
                # |z3| on DVE (bitwise clear of f32 sign), sign(z3) on ACT
                nc.vector.tensor_tensor(a3u[:], z_ps[:].bitcast(U32),
                                        mskA[:], OP.bitwise_and)
                act(sg3[:], z_ps[:], AF.Sign)

        # --- PE: prefill xz of ALL gates of t+1 (fills the ACT/DVE tail) ---
        if t + 1 < T:
            for g in GATE_ORDER:
                pending[g] = emit_xz(t + 1, g, with_stop=False)

        # --- c~ = sign(z3)*(sqrt(1+|z3|)-1); c = f*c + i*c~ ---
        s3 = gp.tile([128, 512], F32, tag="s3")
        act(s3[:], a3u[:].bitcast(F32), AF.Sqrt, bias=1.0)
        i_sg = gp.tile([128, 512], BF16, tag="i_sg")
        nc.gpsimd.tensor_tensor(i_sg[:], sig_i[:], sg3[:], OP.mult)
        s3m1 = gp.tile([128, 512], BF16, tag="s3m1")
        nc.vector.tensor_scalar(s3m1[:], s3[:], 1.0, None, OP.subtract)
        tmp2 = gp.tile([128, 512], BF16, tag="tmp2")
        nc.vector.tensor_tensor(tmp2[:], i_sg[:], s3m1[:], OP.mult)
        nc.vector.tensor_tensor(c_st[:], tmp1[:], tmp2[:], OP.add)

        # --- h = sig_o * sign(c) * (sqrt(1+|c|)-1) ---
        ac = gp.tile([128, 256], U32, tag="ac")
        nc.vector.tensor_tensor(ac[:], c_st[:].bitcast(U32), mskB[:],
                                OP.bitwise_and)
        sc = gp.tile([128, 512], F32, tag="sc")
        act(sc[:], ac[:].bitcast(BF16), AF.Sqrt, bias=1.0)
        cand = gp.tile([128, 256], U32, tag="cand")
        nc.vector.tensor_tensor(cand[:], c_st[:].bitcast(U32), msk[:],
                                OP.bitwise_and)
        osg = gp.tile([128, 256], U32, tag="osg")
        nc.vector.tensor_tensor(osg[:], cand[:], sig_o[:].bitcast(U32),
                                OP.bitwise_xor)
        scm1 = gp.tile([128, 512], BF16, tag="scm1")
        nc.vector.tensor_scalar(scm1[:], sc[:], 1.0, None, OP.subtract)
        h_bf = gp.tile([128, 512], BF16, tag="h_bf")
        nc.vector.tensor_tensor(h_bf[:], scm1[:], osg[:].bitcast(BF16),
                                OP.mult)

        # --- PE: transpose h (4 full 128x128 fold-pair transposes) ---
        t_all = tp.tile([128, 4, 128], BF16, tag="tall")
        hT = ht_pool.tile([128, 4, 128], BF16, tag="hT")
        for j in range(4):
            nc.tensor.transpose(t_all[:, j, :],
                                h_bf[:, j * 128:(j + 1) * 128], id128[:])
        nc.vector.tensor_copy(hT[:], t_all[:])

        # --- PE: FC raw logits (transposed): f_ps = gw.T @ hT ---
        f_ps = fp.tile([C, BL], F32, tag="fps")
        for k in range(KC):
            nc.tensor.matmul(f_ps[:], gw_sb[:, k, :], hT_slice(hT, k),
                             start=(k == 0), stop=(k == KC - 1))
        nc.vector.tensor_copy(logitsT[:, t, :], f_ps[:])

        # --- LN stats (mean/M2 per partition-half; merged post-loop) ---
        nc.vector.bn_stats(stats[:, t, :], h_bf[:])

        hT_prev = hT
        xts.pop(t, None)

    # ================= post-loop finalize =================
    # fold the upper partition half of the stats next to the lower half
    st_lo = sing.tile([BL, T, 6], F32)
    nc.sync.dma_start(st_lo[:], stats[BL:128, :, :])
    up = stats

    # raw logits back to batch-major (batches of 8 steps per PSUM bank)
    raw_bm = sing.tile([BL, T, C], F32)
    for t0 in range(0, T, 8):
        nb = min(8, T - t0)
        lt_ps = lp.tile([BL, 8, C], F32, tag="lt")
        for jj in range(nb):
            nc.tensor.transpose(lt_ps[:, jj, :], logitsT[:, t0 + jj, :],
                                id10[:])
        nc.vector.tensor_copy(raw_bm[:, t0:t0 + nb, :], lt_ps[:, 0:nb, :])


    def f_tile(tag):
        return sp.tile([BL, T], F32, tag=tag, name=tag)

    # sum of the 4 group means and of the 4 squared means, sum of M2s
    msum = f_tile("msum")
    nc.vector.tensor_tensor(msum[:], up[0:BL, :, 1], up[0:BL, :, 4], OP.add)
    t_a = f_tile("t_a")
    nc.vector.tensor_tensor(t_a[:], st_lo[:, :, 1], st_lo[:, :, 4], OP.add)
    nc.vector.tensor_tensor(msum[:], msum[:], t_a[:], OP.add)

    def sq_add(acc, src, first):
        q = f_tile("q")
        nc.vector.tensor_tensor(q[:], src, src, OP.mult)
        if first:
            nc.vector.tensor_copy(acc[:], q[:])
        else:
            nc.vector.tensor_tensor(acc[:], acc[:], q[:], OP.add)

    msq = f_tile("msq")
    sq_add(msq, up[0:BL, :, 1], True)
    sq_add(msq, up[0:BL, :, 4], False)
    sq_add(msq, st_lo[:, :, 1], False)
    sq_add(msq, st_lo[:, :, 4], False)

    m2s = f_tile("m2s")
    nc.vector.tensor_tensor(m2s[:], up[0:BL, :, 2], up[0:BL, :, 5], OP.add)
    nc.vector.tensor_tensor(t_a[:], st_lo[:, :, 2], st_lo[:, :, 5], OP.add)
    nc.vector.tensor_tensor(m2s[:], m2s[:], t_a[:], OP.add)

    mu = f_tile("mu")
    nc.vector.tensor_scalar(mu[:], msum[:], 0.25, None, OP.mult)
    e2 = f_tile("e2")
    nc.vector.tensor_scalar(e2[:], m2s[:], 1.0 / H, None, OP.mult)
    nc.vector.tensor_scalar(t_a[:], msq[:], 0.25, None, OP.mult)
    nc.vector.tensor_tensor(e2[:], e2[:], t_a[:], OP.add)
    var = f_tile("var")
    nc.vector.tensor_tensor(t_a[:], mu[:], mu[:], OP.mult)
    nc.vector.tensor_tensor(var[:], e2[:], t_a[:], OP.subtract)
    sd = f_tile("sd")
    act(sd[:], var[:], AF.Sqrt, bias=eps_sb[:])
    rsig = f_tile("rsig")
    nc.vector.reciprocal(rsig[:], sd[:])
    murs = f_tile("murs")
    nc.vector.tensor_tensor(murs[:], mu[:], rsig[:], OP.mult)

    out_bm = sing.tile([BL, T, C], F32)
    tb = f_tile("tb")
    for c in range(C):
        # out_c = raw_c * rsig - (u_c * mu*rsig - vb_c)
        nc.vector.tensor_scalar(tb[:], murs[:], float(u_vec[c]),
                                float(vb_vec[c]), OP.mult, OP.subtract)
        ta2 = f_tile("ta2")
        nc.vector.tensor_tensor(ta2[:], raw_bm[:, :, c], rsig[:], OP.mult)
        nc.vector.tensor_tensor(out_bm[:, :, c], ta2[:], tb[:], OP.subtract)

    nc.sync.dma_start(dOUT[:], out_bm[:].rearrange("b t c -> b (t c)"))


def build(T=T_FULL, with_bias=False, u_vec=None, vb_vec=None):
    from contextlib import ExitStack

    nc = bacc.Bacc("TRN2", target_bir_lowering=False)
    with tile.TileContext(nc) as tc:
        with ExitStack() as ctx:
            emit(ctx, nc, tc, T, with_bias, u_vec, vb_vec)
    nc.compile()
    return nc


def kernel(x, W, U, b, ln_g, ln_b, fc_w, fc_b, _T=None, _trace=False):
    x = np.asarray(x, dtype=np.float32)
    W = np.asarray(W, dtype=np.float32)
    U = np.asarray(U, dtype=np.float32)
    b = np.asarray(b, dtype=np.float32)
    ln_g = np.asarray(ln_g, dtype=np.float32)
    ln_b = np.asarray(ln_b, dtype=np.float32)
    fc_w = np.asarray(fc_w, dtype=np.float32)
    fc_b = np.asarray(fc_b, dtype=np.float32)

    T = _T or x.shape[1]
    with_bias = bool(np.any(b))

    W_all = np.concatenate([W[g] for g in range(4)], axis=1)  # (H, 4H)
    U_all = np.concatenate([U[g] for g in range(4)], axis=1)
    Wn = np.ascontiguousarray(
        W_all.reshape(KC, 128, G4)).astype(ml_dtypes.bfloat16)
    Un = np.ascontiguousarray(
        U_all.reshape(KC, 128, G4)).astype(ml_dtypes.bfloat16)
    gw = (ln_g[:, None] * fc_w).reshape(KC, 128, C).astype(ml_dtypes.bfloat16)
    u_vec = (ln_g @ fc_w).astype(np.float32)  # (C,)
    vb = (ln_b @ fc_w + fc_b).astype(np.float32)

    common = {"Wn": Wn, "Un": Un, "gw": gw}
    if with_bias:
        b_all = np.concatenate([b[g] for g in range(4)])  # (4H,)
        bb = np.zeros((128, G4), dtype=ml_dtypes.bfloat16)
        bb[0, :] = b_all.astype(ml_dtypes.bfloat16)
        common["bb"] = bb

    in_maps = []
    for ci in range(N_CORES):
        xc = x[ci * BL:(ci + 1) * BL, :T]           # (BL, T, H)
        xT = xc.transpose(1, 2, 0)                   # (T, H, BL)
        xT = np.ascontiguousarray(xT).reshape(T, KC, 128, BL)
        in_maps.append({"xT": xT.astype(ml_dtypes.bfloat16), **common})

    nc = build(T, with_bias, u_vec, vb)
    res = run_bass_kernel_spmd(nc, in_maps, core_ids=list(range(N_CORES)),
                               trace=_trace)
    out = np.concatenate(
        [r["out"].reshape(BL, T, C) for r in res.results], axis=0)
    if _trace:
        kernel.last_exec_time_ns = res.exec_time_ns
    return out
